# revision 1
# baseline (speedup 1.0000x reference)
"""MixHop GNN (3 layers, hops {0,1,2}) on 8 Trainium2 NeuronCores.

Strategy (1D node partition):
 - Nodes padded to NPAD=100352 = 8*12544; core c owns destination rows
   [c*12544, (c+1)*12544).
 - gcn_norm edge weight w_e = dinv[src]*dinv[dst] is factorized:
   gather tables are pre-scaled by dinv[src] ("scale-in-table"), spmm
   output is post-scaled by dinv[dst] (per-partition tensor_scalar).
 - SpMM = batched dma_gather of source rows (bf16, 256B each) +
   selection-matrix matmul: for each 128-edge chunk, Sel[e,d] =
   (ldest[e]==d) built by one DVE is_equal; PE accumulates
   Sel.T @ G into the [128 dest x 128 feat] PSUM tile of the dest tile.
 - Edges are sorted by (dest tile, src parity); each (tile,parity)
   group is padded to K5 chunks of 128 (uniform across cores so one
   SPMD program serves all 8). Parity split keeps int16 gather indices
   in range (idx = src>>1, elem_step = 2 rows).
 - Layer 1 uses the commuted form out_j = (A^j x) W + (A^j 1) b so the
   (already replicated) x input is the first gather table: only ONE
   AllGather in layer 1. Layers 2/3: transform-then-aggregate like the
   reference; 3 AllGathers each (t1, t2, A(t2)).
 - Dense transforms run feature-major (lhsT = W block, rhs = h^T tile,
   512-node moving dim); biases are K=1 outer-product matmuls; BN+ReLU
   is fused into scalar-engine activation(Relu, scale=A, bias=B) with
   per-feature (= per-partition) A/B; BN stats are free-axis reduces +
   a tiny AllReduce.
 - All matmul operands bf16 (fp32 PE matmul is 4x slower); PSUM
   accumulation fp32; final output written fp32.
"""
import os
import numpy as np
import ml_dtypes

import concourse.bass as bass
import concourse.mybir as mybir
import concourse.tile as tile
from concourse import library_config
from concourse.bass_utils import run_bass_kernel_spmd

bf16 = mybir.dt.bfloat16
f32 = mybir.dt.float32
i16 = mybir.dt.int16
BF = ml_dtypes.bfloat16

N = 100000
NC = 8
P = 128
D = 128
SHARD = 12544
NPAD = SHARD * NC
TPC = SHARD // P          # 98 dest tiles per core
GROUP = 14                # dest tiles per gather group
NGROUPS = TPC // GROUP    # 7
GSZ = 512                 # transform moving-dim group
RQ = 25088                # gather source quarter-range rows (NPAD/4)
GBUFS = int(os.environ.get("KGBUFS", "2"))
EPS = 1e-5

AOP = mybir.AluOpType
AF = mybir.ActivationFunctionType


# ---------------------------------------------------------------- wait split
_SKIP_WAITSPLIT = (mybir.InstEventSemaphore,)


def _split_excess_waits(nc, keep=1):
    """This walrus build allows only 1 embedded sync-wait on most
    instructions; hoist extras into EventSemaphore insts placed before."""
    n = 0
    uid = [0]
    for fn in nc.m.functions:
        for blk in fn.blocks:
            insts = list(blk.instructions)
            out = []
            for inst in insts:
                si = inst.sync_info
                if (si is not None and si.on_wait and len(si.on_wait) > keep
                        and not isinstance(inst, _SKIP_WAITSPLIT)):
                    waits = list(si.on_wait)
                    extra, rest = waits[:-keep], waits[-keep:]
                    for w in extra:
                        uid[0] += 1
                        out.append(mybir.InstEventSemaphore(
                            name=f"evws_{uid[0]}",
                            engine=inst.engine,
                            ins=[], outs=[],
                            sync_info=mybir.SyncInfo(on_wait=[w], on_update=[]),
                        ))
                        n += 1
                    inst.sync_info = mybir.SyncInfo(
                        on_wait=rest, on_update=list(si.on_update or []))
                out.append(inst)
            if len(out) != len(insts):
                blk.instructions = out
    return n


# ---------------------------------------------------------------- host prep
def _host_prep(x, edge_index):
    row = edge_index[0].astype(np.int64)
    col = edge_index[1].astype(np.int64)
    deg = np.bincount(col, minlength=N).astype(np.float64)
    dinv = np.where(deg > 0, 1.0 / np.sqrt(np.maximum(deg, 1.0)), 0.0)
    s1 = dinv * np.bincount(col, weights=dinv[row], minlength=N)
    s2 = dinv * np.bincount(col, weights=dinv[row] * s1[row], minlength=N)

    dinv_pad = np.zeros(NPAD, np.float64)
    dinv_pad[:N] = dinv
    x_pad = np.zeros((NPAD, D), np.float32)
    x_pad[:N] = x
    xtbl = (dinv_pad[:, None] * x_pad).astype(BF)      # L1 gather table
    s1_pad = np.zeros(NPAD, np.float32)
    s1_pad[:N] = s1
    s2_pad = np.zeros(NPAD, np.float32)
    s2_pad[:N] = s2
    ones_pad = np.zeros(NPAD, np.float32)
    ones_pad[:N] = 1.0

    # per-core edge structures
    per = []
    maxcnt = 0
    for c in range(NC):
        lo, hi = c * SHARD, (c + 1) * SHARD
        m = (col >= lo) & (col < hi)
        r, cc = row[m], col[m] - lo
        t = cc >> 7
        res = r // RQ                  # source quarter-range
        key = t * 4 + res
        order = np.argsort(key, kind="stable")
        key_s = key[order]
        cnt = np.bincount(key_s, minlength=TPC * 4)
        maxcnt = max(maxcnt, int(cnt.max()))
        per.append((r[order], cc[order], key_s, cnt))

    K5 = max(1, -(-maxcnt // P))    # chunks per (tile,quarter)
    CPT = 4 * K5
    NCHUNK = TPC * CPT
    SEGIDX = GROUP * K5 * P
    IDXTOT = NGROUPS * 4 * SEGIDX

    cores = []
    for c in range(NC):
        lo, hi = c * SHARD, (c + 1) * SHARD
        r_s, cc_s, key_s, cnt = per[c]
        starts = np.zeros(TPC * 4, np.int64)
        starts[1:] = np.cumsum(cnt)[:-1]
        rank = np.arange(len(key_s)) - starts[key_s]
        base = (key_s // 4) * CPT * P + (key_s % 4) * K5 * P
        slot = base + rank
        idx_arr = np.zeros(NCHUNK * P, np.int16)
        ld_arr = np.full(NCHUNK * P, -1.0, np.float32)
        idx_arr[slot] = (r_s % RQ).astype(np.int16)
        ld_arr[slot] = (cc_s & 127).astype(np.float32)

        idx_slots = idx_arr.reshape(TPC, 4, K5 * P)
        idx16 = np.zeros((P, IDXTOT // 16), np.int16)
        segc = SEGIDX // 16
        for g in range(NGROUPS):
            for res in range(4):
                seg = idx_slots[g * GROUP:(g + 1) * GROUP, res, :].ravel()
                blk = seg.reshape(-1, 16).T          # [16, SEGIDX/16]
                i0 = (g * 4 + res) * segc
                idx16[:, i0:i0 + segc] = np.tile(blk, (8, 1))

        ldest = ld_arr.reshape(NCHUNK, P).T.astype(BF)   # [P, NCHUNK]
        d1 = dinv_pad[lo:hi].reshape(TPC, P).T.astype(np.float32)
        d2 = (dinv_pad[lo:hi] ** 2).reshape(TPC, P).T.astype(np.float32)
        xt = x_pad[lo:hi].T.astype(BF)                   # [P, SHARD]
        srow = np.concatenate([s1_pad[lo:hi], s2_pad[lo:hi]])[None, :]
        onesr = ones_pad[lo:hi][None, :]
        cores.append(dict(idx16=idx16, ldest=ldest, d1=d1, d2=d2, xt=xt,
                          srow=srow.astype(BF), onesr=onesr.astype(BF)))
    return xtbl, cores, K5


def _pack_consts(core, K5, W0, b0, W1, b1, W2, b2, bn_g, bn_b):
    CPT = 4 * K5
    NCHUNK = TPC * CPT
    iota = np.tile(np.arange(P, dtype=np.float32), (P, CPT)).astype(BF)
    ident = np.eye(P, dtype=np.float32)
    w0c = np.concatenate([W0[j] for j in range(3)], axis=1)          # [P,3P]
    blocks = []
    for W in (W1, W2):
        for j in range(3):
            for b in range(3):
                blocks.append(W[j][b * P:(b + 1) * P, :])
    w12c = np.concatenate(blocks, axis=1)                            # [P,18P]
    cb = np.concatenate([iota, core["ldest"], ident.astype(BF),
                         w0c.astype(BF), w12c.astype(BF)], axis=1)
    # rows: biases [9P] | s rows [2*SHARD] | ones [SHARD]
    br = np.concatenate([b0.reshape(1, -1), b1.reshape(1, -1),
                         b2.reshape(1, -1)], axis=1)                 # [1,9P]
    cr = np.concatenate([br.astype(np.float32), core["srow"].astype(np.float32),
                         core["onesr"].astype(np.float32)], axis=1).astype(BF)
    # f32: dinv1 | dinv2 | bng(6) | bnb(6) | identf32(128)
    bng = np.stack([bn_g[l].reshape(3, P).T for l in range(2)], axis=0)
    bnb = np.stack([bn_b[l].reshape(3, P).T for l in range(2)], axis=0)
    cf = np.concatenate([core["d1"], core["d2"],
                         bng[0], bng[1], bnb[0], bnb[1], ident], axis=1)
    return cb.astype(BF), cr, cf.astype(np.float32)


# ---------------------------------------------------------------- device
def _build(K5, phase="full", for_sim=False):
    CPT = 4 * K5
    NCHUNK = TPC * CPT
    SEGIDX = GROUP * K5 * P
    IDXTOT = NGROUPS * 4 * SEGIDX
    SEGC = SEGIDX // 16
    NCB = CPT * P + NCHUNK + P + 3 * P + 18 * P
    NCR = 9 * P + 2 * SHARD + SHARD
    NCF = TPC * 2 + 12 + P
    NG = (SHARD + GSZ - 1) // GSZ      # transform groups (25)

    nc = bass.Bass(num_devices=NC)
    xtbl_t = nc.dram_tensor("xtbl", [NPAD, P], bf16, kind="ExternalInput")
    xt_t = nc.dram_tensor("xt", [P, SHARD], bf16, kind="ExternalInput")
    idx_t = nc.dram_tensor("idx", [P, IDXTOT // 16], i16, kind="ExternalInput")
    cb_t = nc.dram_tensor("cb", [P, NCB], bf16, kind="ExternalInput")
    cr_t = nc.dram_tensor("cr", [1, NCR], bf16, kind="ExternalInput")
    cf_t = nc.dram_tensor("cf", [P, NCF], f32, kind="ExternalInput")
    out_t = nc.dram_tensor("out", [SHARD, 3 * P], f32, kind="ExternalOutput")

    with tile.TileContext(nc) as tc:
        with (
            tc.tile_pool(name="const", bufs=1) as cpool,
            tc.tile_pool(name="gath", bufs=2) as gpool,
            tc.tile_pool(name="sel", bufs=3) as spool,
            tc.tile_pool(name="work", bufs=3) as wpool,
            tc.tile_pool(name="hb", bufs=2) as hpool,
            tc.tile_pool(name="stat", bufs=1) as stpool,
            tc.tile_pool(name="psp", bufs=3, space="PSUM") as ps_sp,
            tc.tile_pool(name="ptp", bufs=2, space="PSUM") as ps_tp,
            tc.tile_pool(name="ptf", bufs=2, space="PSUM") as ps_tf,
            tc.tile_pool(name="dram", bufs=1, space="DRAM") as dpool,
        ):
            nc.gpsimd.load_library(library_config.mlp)

            idx_sb = cpool.tile([P, IDXTOT // 16], i16)
            nc.sync.dma_start(idx_sb[:], idx_t[:])
            cb_sb = cpool.tile([P, NCB], bf16)
            nc.sync.dma_start(cb_sb[:], cb_t[:])
            bias_sb = cpool.tile([1, 9 * P], bf16)
            nc.sync.dma_start(bias_sb[:], cr_t[:1, :9 * P])
            cf_sb = cpool.tile([P, NCF], f32)
            nc.sync.dma_start(cf_sb[:], cf_t[:])

            o = 0
            iota_sb = cb_sb[:, o:o + CPT * P]; o += CPT * P
            ldest_sb = cb_sb[:, o:o + NCHUNK]; o += NCHUNK
            ident_sb = cb_sb[:, o:o + P]; o += P
            w0_sb = cb_sb[:, o:o + 3 * P]; o += 3 * P
            w12_sb = cb_sb[:, o:o + 18 * P]

            def wblk(l, j, b):  # layer l in {1,2}
                i = ((l - 1) * 9 + j * 3 + b) * P
                return w12_sb[:, i:i + P]

            def brow(l, j):     # bias row [1,P], layer l in {0,1,2}
                i = (l * 3 + j) * P
                return bias_sb[:1, i:i + P]

            def load_row(which, n0, w, tag):
                # which: 0=s1, 1=s2, 2=ones; stream [1,w] slice from DRAM
                i = 9 * P + which * SHARD + n0
                rt = wpool.tile([1, GSZ], bf16, name="row", tag=tag)
                nc.sync.dma_start(rt[:1, :w], cr_t[:1, i:i + w])
                return rt[:1, :w]

            dinv1_sb = cf_sb[:, 0:TPC]
            dinv2_sb = cf_sb[:, TPC:2 * TPC]

            def bng(l, b):
                return cf_sb[:, 2 * TPC + l * 3 + b:2 * TPC + l * 3 + b + 1]

            def bnb(l, b):
                i = 2 * TPC + 6 + l * 3 + b
                return cf_sb[:, i:i + 1]

            identf_sb = cf_sb[:, 2 * TPC + 12:2 * TPC + 12 + P]

            # DRAM staging
            y1d = dpool.tile([SHARD, P], bf16, name="y1d")
            y2d = dpool.tile([SHARD, P], bf16, name="y2d")
            rawh = {}
            for l in (1, 2):
                for b in range(3):
                    rawh[(l, b)] = dpool.tile([P, SHARD], bf16,
                                              name=f"rawh{l}{b}")
            agbuf = [dpool.tile([SHARD, P], bf16, name=f"agin{i}", bufs=1)
                     for i in range(2)]
            _tbl_n = [0]

            def new_tbl():
                _tbl_n[0] += 1
                return dpool.tile([NPAD, P], bf16, name=f"tbl{_tbl_n[0]}",
                                  bufs=1, addr_space="Shared")
            arin = [dpool.tile([P, 8], f32, name=f"arin{l}") for l in range(2)]
            arout = [dpool.tile([P, 8], f32, name=f"arout{l}",
                                addr_space="Shared") for l in range(2)]

            def quarter_views(tensor_ap):
                return [tensor_ap[q * RQ:(q + 1) * RQ, :] for q in range(4)]

            iota3 = iota_sb.rearrange("p (c e) -> p c e", e=P)
            segreg = nc.gpsimd.to_reg(SEGIDX)

            # -------------------------------------------------- spmm unit
            def spmm_unit(tbl_ap, sink, uname):
                tv = quarter_views(tbl_ap)
                for g in range(NGROUPS):
                    gt = []
                    for res in range(4):
                        gtile = gpool.tile([P, SEGIDX], bf16,
                                           name=f"g{uname}", tag=f"g{res}",
                                           bufs=GBUFS)
                        seg = (g * 4 + res) * SEGC
                        if os.environ.get("KNOGATHER"):
                            nc.vector.memset(gtile[:], 0.25)
                        else:
                            nc.gpsimd.dma_gather(
                                out_ap=gtile.rearrange("p (c e) -> p c e", e=P),
                                in_ap=tv[res],
                                idxs_ap=idx_sb[:, seg:seg + SEGC],
                                num_idxs=SEGIDX,
                                num_idxs_reg=segreg,
                                elem_size=P,
                                single_packet=False,
                            )
                        gt.append(gtile.rearrange("p (c e) -> p c e", e=P))
                    for tt in range(GROUP):
                        t = g * GROUP + tt
                        sel = spool.tile([P, CPT * P], bf16, name=f"s{uname}",
                                         tag="sel")
                        nc.vector.tensor_tensor(
                            out=sel.rearrange("p (c e) -> p c e", e=P),
                            in0=iota3,
                            in1=ldest_sb[:, t * CPT:(t + 1) * CPT, None]
                                .to_broadcast([P, CPT, P]),
                            op=AOP.is_equal)
                        ps = ps_sp.tile([P, P], f32, name=f"p{uname}",
                                        tag="sp", space="PSUM")
                        for cidx in range(CPT):
                            res, k = divmod(cidx, K5)
                            nc.tensor.matmul(
                                ps[:],
                                lhsT=sel[:, cidx * P:(cidx + 1) * P],
                                rhs=gt[res][:, tt * K5 + k, :],
                                start=(cidx == 0), stop=(cidx == CPT - 1))
                        sink(t, ps)

            # sinks ------------------------------------------------------
            def mk_sink_dual(dst_h, dst_tbl):
                # h-value (dinv) to dst_h rows; table value (dinv^2) to AG in
                def sink(t, ps):
                    a = wpool.tile([P, P], bf16, name="sh", tag="sh")
                    nc.vector.tensor_scalar(
                        out=a[:], in0=ps[:], scalar1=dinv1_sb[:, t:t + 1],
                        scalar2=None, op0=AOP.mult)
                    nc.sync.dma_start(dst_h[t * P:(t + 1) * P, :], a[:])
                    b = wpool.tile([P, P], bf16, name="st", tag="st")
                    nc.vector.tensor_scalar(
                        out=b[:], in0=ps[:], scalar1=dinv2_sb[:, t:t + 1],
                        scalar2=None, op0=AOP.mult)
                    nc.sync.dma_start(dst_tbl[t * P:(t + 1) * P, :], b[:])
                return sink

            def mk_sink_h(dst_h):
                def sink(t, ps):
                    a = wpool.tile([P, P], bf16, name="sh2", tag="sh")
                    nc.vector.tensor_scalar(
                        out=a[:], in0=ps[:], scalar1=dinv1_sb[:, t:t + 1],
                        scalar2=None, op0=AOP.mult)
                    nc.sync.dma_start(dst_h[t * P:(t + 1) * P, :], a[:])
                return sink

            def mk_sink_tbl(dst_tbl):
                def sink(t, ps):
                    b = wpool.tile([P, P], bf16, name="st2", tag="st")
                    nc.vector.tensor_scalar(
                        out=b[:], in0=ps[:], scalar1=dinv2_sb[:, t:t + 1],
                        scalar2=None, op0=AOP.mult)
                    nc.sync.dma_start(dst_tbl[t * P:(t + 1) * P, :], b[:])
                return sink

            def mk_sink_block(l, b, stS, stQ):
                # out_j = dinv*psum -> transpose -> stats + rawh[l][b]
                def sink(t, ps):
                    a = wpool.tile([P, P], bf16, name="sb", tag="sh")
                    nc.vector.tensor_scalar(
                        out=a[:], in0=ps[:], scalar1=dinv1_sb[:, t:t + 1],
                        scalar2=None, op0=AOP.mult)
                    pst = ps_tp.tile([P, P], bf16, name="pt", tag="tp",
                                     space="PSUM")
                    nc.tensor.transpose(pst[:], a[:], ident_sb)
                    c = wpool.tile([P, P], bf16, name="cb2", tag="st")
                    nc.scalar.activation(out=c[:], in_=pst[:], func=AF.Copy)
                    nc.sync.dma_start(rawh[(l, b)][:, t * P:(t + 1) * P], c[:])
                    nc.vector.reduce_sum(out=stS[:, t:t + 1], in_=pst[:],
                                         axis=mybir.AxisListType.X)
                    sq = wpool.tile([P, P], f32, name="sq", tag="sq")
                    nc.vector.tensor_tensor(out=sq[:], in0=c[:], in1=c[:],
                                            op=AOP.mult)
                    nc.vector.reduce_sum(out=stQ[:, t:t + 1], in_=sq[:],
                                         axis=mybir.AxisListType.X)
                return sink

            def mk_sink_out(colbase):
                def sink(t, ps):
                    a = wpool.tile([P, P], f32, name="so", tag="so")
                    nc.vector.tensor_scalar(
                        out=a[:], in0=ps[:], scalar1=dinv1_sb[:, t:t + 1],
                        scalar2=None, op0=AOP.mult)
                    nc.sync.dma_start(
                        out_t[t * P:(t + 1) * P, colbase:colbase + P], a[:])
                return sink

            def allgather(src):
                dst = new_tbl()
                nc.gpsimd.collective_compute(
                    "AllGather", AOP.bypass,
                    replica_groups=[list(range(NC))],
                    ins=[src[:]], outs=[dst[:]])
                return dst

            # debug sink: write dinv*psum as f32 straight to OUT cols 0:128
            def mk_sink_dbg(colbase):
                def sink(t, ps):
                    a = wpool.tile([P, P], f32, name="sd", tag="so")
                    nc.vector.tensor_scalar(
                        out=a[:], in0=ps[:], scalar1=dinv1_sb[:, t:t + 1],
                        scalar2=None, op0=AOP.mult)
                    nc.sync.dma_start(
                        out_t[t * P:(t + 1) * P, colbase:colbase + P], a[:])
                return sink

            if phase == "u1":
                spmm_unit(xtbl_t[:], mk_sink_dbg(0), "u1")
            elif phase == "u2":
                spmm_unit(xtbl_t[:], mk_sink_dual(y1d, agbuf[0]), "u1")
                tbx = allgather(agbuf[0])
                spmm_unit(tbx[:], mk_sink_dbg(0), "u2")
            if phase == "full":
                # ============================================== LAYER 1
                st = {}
                for key, ncols in (("S0", NG), ("Q0", NG), ("S1", NG), ("Q1", NG),
                                   ("S2", NG), ("Q2", NG)):
                    st[(1, key)] = stpool.tile([P, NG], f32, name=f"st1{key}")

                spmm_unit(xtbl_t[:], mk_sink_dual(y1d, agbuf[0]), "u1")
                tb = allgather(agbuf[0])
                spmm_unit(tb[:], mk_sink_h(y2d), "u2")

                # L1 transforms: feature-major, x^T resident; y1/y2 transposed in
                for grp in range(NG):
                    n0 = grp * GSZ
                    w = min(GSZ, SHARD - n0)
                    nq = w // P
                    # hop 0
                    xg = hpool.tile([P, GSZ], bf16, name="xg", tag="xg")
                    nc.sync.dma_start(xg[:, :w], xt_t[:, n0:n0 + w])
                    ps0 = ps_tf.tile([P, GSZ], f32, name="tf0", tag="tf",
                                     space="PSUM")
                    nc.tensor.matmul(ps0[:, :w], lhsT=w0_sb[:, 0:P],
                                     rhs=xg[:, :w], start=True,
                                     stop=False)
                    nc.tensor.matmul(ps0[:, :w], lhsT=brow(0, 0),
                                     rhs=load_row(2, n0, w, "ro"), start=False,
                                     stop=True)
                    nc.vector.reduce_sum(out=st[(1, "S0")][:, grp:grp + 1],
                                         in_=ps0[:, :w], axis=mybir.AxisListType.X)
                    cp = hpool.tile([P, GSZ], bf16, name="cpt", tag="cpt")
                    nc.scalar.activation(out=cp[:, :w], in_=ps0[:, :w],
                                         func=AF.Copy)
                    sq = wpool.tile([P, GSZ], f32, name="sqt", tag="sqt")
                    nc.vector.tensor_tensor(out=sq[:, :w], in0=cp[:, :w],
                                            in1=cp[:, :w], op=AOP.mult)
                    nc.vector.reduce_sum(out=st[(1, "Q0")][:, grp:grp + 1],
                                         in_=sq[:, :w], axis=mybir.AxisListType.X)
                    nc.sync.dma_start(rawh[(1, 0)][:, n0:n0 + w], cp[:, :w])
                    # hops 1,2 from y1d/y2d (node-major staging -> transpose)
                    for hop, ydram in ((1, y1d), (2, y2d)):
                        yT = hpool.tile([P, GSZ], bf16, name="yT", tag=f"yT{hop}")
                        for q in range(nq):
                            ld = wpool.tile([P, P], bf16, name="ldy", tag="ldy")
                            nc.sync.dma_start(
                                ld[:], ydram[n0 + q * P:n0 + (q + 1) * P, :])
                            pst = ps_tp.tile([P, P], bf16, name="pty", tag="tp",
                                             space="PSUM")
                            nc.tensor.transpose(pst[:], ld[:], ident_sb)
                            nc.vector.tensor_copy(out=yT[:, q * P:(q + 1) * P],
                                                  in_=pst[:])
                        ps1 = ps_tf.tile([P, GSZ], f32, name="tf1", tag="tf",
                                         space="PSUM")
                        nc.tensor.matmul(ps1[:, :w], lhsT=w0_sb[:, hop * P:(hop + 1) * P],
                                         rhs=yT[:, :w], start=True, stop=False)
                        nc.tensor.matmul(ps1[:, :w], lhsT=brow(0, hop),
                                         rhs=load_row(hop - 1, n0, w, "rs"),
                                         start=False, stop=True)
                        nc.vector.reduce_sum(
                            out=st[(1, f"S{hop}")][:, grp:grp + 1],
                            in_=ps1[:, :w], axis=mybir.AxisListType.X)
                        cp2 = hpool.tile([P, GSZ], bf16, name="cpt2", tag="cpt")
                        nc.scalar.activation(out=cp2[:, :w], in_=ps1[:, :w],
                                             func=AF.Copy)
                        sq2 = wpool.tile([P, GSZ], f32, name="sqt2", tag="sqt")
                        nc.vector.tensor_tensor(out=sq2[:, :w], in0=cp2[:, :w],
                                                in1=cp2[:, :w], op=AOP.mult)
                        nc.vector.reduce_sum(
                            out=st[(1, f"Q{hop}")][:, grp:grp + 1],
                            in_=sq2[:, :w], axis=mybir.AxisListType.X)
                        nc.sync.dma_start(rawh[(1, hop)][:, n0:n0 + w],
                                          cp2[:, :w])

                # BN stats AllReduce + A/B for layer-1 output normalization
                def bn_reduce_and_AB(l, stt, tag):
                    ar = wpool.tile([P, 8], f32, name=f"ar{l}", tag="ar")
                    for b in range(3):
                        nc.vector.reduce_sum(out=ar[:, b:b + 1],
                                             in_=stt[f"S{b}"][:],
                                             axis=mybir.AxisListType.X)
                        nc.vector.reduce_sum(out=ar[:, 3 + b:4 + b],
                                             in_=stt[f"Q{b}"][:],
                                             axis=mybir.AxisListType.X)
                    nc.sync.dma_start(arin[l][:], ar[:])
                    nc.gpsimd.collective_compute(
                        "AllReduce", AOP.add, replica_groups=[list(range(NC))],
                        ins=[arin[l][:]], outs=[arout[l][:]])
                    g = wpool.tile([P, 8], f32, name=f"arg{l}", tag="ar")
                    nc.sync.dma_start(g[:], arout[l][:])
                    A = stpool.tile([P, 3], f32, name=f"A{l}")
                    B = stpool.tile([P, 3], f32, name=f"B{l}")
                    mu = wpool.tile([P, 3], f32, name=f"mu{l}", tag="mu")
                    va = wpool.tile([P, 3], f32, name=f"va{l}", tag="mu")
                    nc.vector.tensor_scalar(out=mu[:], in0=g[:, 0:3],
                                            scalar1=1.0 / N, scalar2=None,
                                            op0=AOP.mult)
                    nc.vector.tensor_scalar(out=va[:], in0=g[:, 3:6],
                                            scalar1=1.0 / N, scalar2=None,
                                            op0=AOP.mult)
                    musq = wpool.tile([P, 3], f32, name=f"ms{l}", tag="mu")
                    nc.vector.tensor_tensor(out=musq[:], in0=mu[:], in1=mu[:],
                                            op=AOP.mult)
                    nc.vector.tensor_tensor(out=va[:], in0=va[:], in1=musq[:],
                                            op=AOP.subtract)
                    # rsqrt(var+eps) = 1/sqrt(var+eps); Rsqrt ACT is banned
                    ve = wpool.tile([P, 3], f32, name=f"ve{l}", tag="mu")
                    nc.vector.tensor_scalar(out=ve[:], in0=va[:],
                                            scalar1=float(EPS), scalar2=None,
                                            op0=AOP.add)
                    sq_ = wpool.tile([P, 3], f32, name=f"sv{l}", tag="mu")
                    nc.scalar.activation(out=sq_[:], in_=ve[:], func=AF.Sqrt)
                    rs = wpool.tile([P, 3], f32, name=f"rs{l}", tag="mu")
                    nc.vector.reciprocal(out=rs[:], in_=sq_[:])
                    muA = wpool.tile([P, 3], f32, name=f"ma{l}", tag="mu")
                    for b in range(3):
                        nc.vector.tensor_tensor(out=A[:, b:b + 1],
                                                in0=rs[:, b:b + 1],
                                                in1=bng(l, b), op=AOP.mult)
                        nc.vector.tensor_tensor(out=muA[:, b:b + 1],
                                                in0=mu[:, b:b + 1],
                                                in1=A[:, b:b + 1], op=AOP.mult)
                        nc.vector.tensor_tensor(out=B[:, b:b + 1],
                                                in0=bnb(l, b),
                                                in1=muA[:, b:b + 1],
                                                op=AOP.subtract)
                    return A, B

                stt1 = {k: st[(1, k)] for k in
                        ("S0", "Q0", "S1", "Q1", "S2", "Q2")}
                A1, B1 = bn_reduce_and_AB(0, stt1, "l1")

                # ============================================== LAYERS 2,3
                def transform_layer(l, A, B, final):
                    """l in {1,2} selects W/b set (layer l+1); reads rawh[l]."""
                    for grp in range(NG):
                        n0 = grp * GSZ
                        w = min(GSZ, SHARD - n0)
                        nq = w // P
                        hb = []
                        for b in range(3):
                            raw = hpool.tile([P, GSZ], bf16, name="raw",
                                             tag=f"raw{b}")
                            nc.sync.dma_start(raw[:, :w],
                                              rawh[(l, b)][:, n0:n0 + w])
                            h = hpool.tile([P, GSZ], bf16, name="hh",
                                           tag=f"h{b}")
                            nc.scalar.activation(out=h[:, :w], in_=raw[:, :w],
                                                 func=AF.Relu,
                                                 bias=B[:, b:b + 1],
                                                 scale=A[:, b:b + 1])
                            hb.append(h)
                        for j in range(3):
                            ps = ps_tf.tile([P, GSZ], f32, name="tfj", tag="tf",
                                            space="PSUM")
                            for b in range(3):
                                nc.tensor.matmul(ps[:, :w], lhsT=wblk(l, j, b),
                                                 rhs=hb[b][:, :w],
                                                 start=(b == 0), stop=False)
                            nc.tensor.matmul(ps[:, :w], lhsT=brow(l, j),
                                             rhs=load_row(2, n0, w, "ro"),
                                             start=False, stop=True)
                            if j == 0 and not final:
                                nc.vector.reduce_sum(
                                    out=st[(2, "S0")][:, grp:grp + 1],
                                    in_=ps[:, :w], axis=mybir.AxisListType.X)
                                cp = hpool.tile([P, GSZ], bf16, name="cpj",
                                                tag="cpt")
                                nc.scalar.activation(out=cp[:, :w],
                                                     in_=ps[:, :w], func=AF.Copy)
                                sq = wpool.tile([P, GSZ], f32, name="sqj",
                                                tag="sqt")
                                nc.vector.tensor_tensor(
                                    out=sq[:, :w], in0=cp[:, :w], in1=cp[:, :w],
                                    op=AOP.mult)
                                nc.vector.reduce_sum(
                                    out=st[(2, "Q0")][:, grp:grp + 1],
                                    in_=sq[:, :w], axis=mybir.AxisListType.X)
                                nc.sync.dma_start(rawh[(2, 0)][:, n0:n0 + w],
                                                  cp[:, :w])
                            elif j == 0 and final:
                                # out0 -> transpose to node-major f32 -> OUT
                                for q in range(nq):
                                    cpf = wpool.tile([P, P], f32, name="cpf",
                                                     tag="cpf")
                                    nc.vector.tensor_copy(
                                        out=cpf[:],
                                        in_=ps[:, q * P:(q + 1) * P])
                                    pst = ps_tp.tile([P, P], f32, name="ptf",
                                                     tag="tpf", space="PSUM",
                                                     bufs=1)
                                    nc.tensor.transpose(pst[:], cpf[:],
                                                        identf_sb)
                                    of = wpool.tile([P, P], f32, name="of",
                                                    tag="cpf")
                                    nc.scalar.activation(out=of[:], in_=pst[:],
                                                         func=AF.Copy)
                                    nc.sync.dma_start(
                                        out_t[n0 + q * P:n0 + (q + 1) * P, 0:P],
                                        of[:])
                            else:
                                # t_j -> transpose -> dinv-scale -> AG input
                                dst = agbuf[j - 1]
                                for q in range(nq):
                                    cpb = wpool.tile([P, P], bf16, name="cpb",
                                                     tag="cpb")
                                    nc.vector.tensor_copy(
                                        out=cpb[:],
                                        in_=ps[:, q * P:(q + 1) * P])
                                    pst = ps_tp.tile([P, P], bf16, name="ptb",
                                                     tag="tp", space="PSUM")
                                    nc.tensor.transpose(pst[:], cpb[:], ident_sb)
                                    tt = n0 // P + q
                                    ob = wpool.tile([P, P], bf16, name="ob",
                                                    tag="cpb")
                                    nc.vector.tensor_scalar(
                                        out=ob[:], in0=pst[:],
                                        scalar1=dinv1_sb[:, tt:tt + 1],
                                        scalar2=None, op0=AOP.mult)
                                    nc.sync.dma_start(
                                        dst[n0 + q * P:n0 + (q + 1) * P, :],
                                        ob[:])

                # ---------- layer 2
                for key in ("S0", "Q0", "S1", "Q1", "S2", "Q2"):
                    ncols = NG if key in ("S0", "Q0") else TPC
                    st[(2, key)] = stpool.tile([P, ncols], f32, name=f"st2{key}")
                transform_layer(1, A1, B1, final=False)
                tb1 = allgather(agbuf[0])     # ts1
                tb2 = allgather(agbuf[1])     # ts2
                spmm_unit(tb1[:],
                          mk_sink_block(2, 1, st[(2, "S1")], st[(2, "Q1")]), "v1")
                spmm_unit(tb2[:], mk_sink_tbl(agbuf[0]), "v2")
                tb3 = allgather(agbuf[0])     # us
                spmm_unit(tb3[:],
                          mk_sink_block(2, 2, st[(2, "S2")], st[(2, "Q2")]), "v3")
                stt2 = {k: st[(2, k)] for k in
                        ("S0", "Q0", "S1", "Q1", "S2", "Q2")}
                A2, B2 = bn_reduce_and_AB(1, stt2, "l2")

                # ---------- layer 3 (final: no BN on outputs)
                transform_layer(2, A2, B2, final=True)
                tb4 = allgather(agbuf[0])     # ts1
                tb5 = allgather(agbuf[1])     # ts2
                spmm_unit(tb4[:], mk_sink_out(P), "w1")
                spmm_unit(tb5[:], mk_sink_tbl(agbuf[0]), "w2")
                tb6 = allgather(agbuf[0])     # us
                spmm_unit(tb6[:], mk_sink_out(2 * P), "w3")

    if not for_sim:
        _split_excess_waits(nc)
        mybir.codegen_inst_isa_subclasses(nc)
    return nc


_CACHE = {}


def kernel(x, edge_index, W0, b0, W1, b1, W2, b2, bn_gamma, bn_beta):
    x = np.asarray(x, np.float32)
    edge_index = np.asarray(edge_index)
    xtbl, cores, K5 = _host_prep(x, edge_index)

    W0 = np.asarray(W0, np.float32)
    W1 = np.asarray(W1, np.float32)
    W2 = np.asarray(W2, np.float32)
    b0 = np.asarray(b0, np.float32)
    b1 = np.asarray(b1, np.float32)
    b2 = np.asarray(b2, np.float32)
    bn_g = np.asarray(bn_gamma, np.float32)
    bn_b = np.asarray(bn_beta, np.float32)

    in_maps = []
    for c in range(NC):
        cb, cr, cf = _pack_consts(cores[c], K5, W0, b0, W1, b1, W2, b2,
                                  bn_g, bn_b)
        in_maps.append(dict(
            xtbl=xtbl, xt=cores[c]["xt"], idx=cores[c]["idx16"],
            cb=cb, cr=cr, cf=cf))

    phase = os.environ.get("KPHASE", "full")
    if (K5, phase) not in _CACHE:
        _CACHE[(K5, phase)] = _build(K5, phase)
    nc = _CACHE[(K5, phase)]
    trace = bool(os.environ.get("KERNEL_TRACE"))
    res = run_bass_kernel_spmd(nc, in_maps, core_ids=list(range(NC)),
                               trace=trace)
    global last_result
    last_result = res
    out = np.concatenate([r["out"] for r in res.results], axis=0)
    return out[:N].astype(np.float32)


last_result = None

